# revision 11
# baseline (speedup 1.0000x reference)
"""Ewald real-space potential kernel for Trainium2 (8 NeuronCores, SPMD).

pot = C * sum_{i<j} q_i q_j erf(d_ij/sqrt(2)) / d_ij,  C = 90.0474/(2*pi).

V4 design:

  * DEVICE computes only the dense far-field rinv sum in a TRANSPOSED
    layout: per core, 20 j-blocks of [128 j-partitions x <=512 i-cols]:
      PE  : d2 via K=16 fp16 hi/lo split matmuls into 5 PSUM super-tiles
            (1536/2048 cols, two alternating pools of 3+4 banks)
      ACT : rinv = Abs_reciprocal_sqrt(d2), ONE activate per super (fp16)
      DVE : strict triangular boundary masks on the 8 edge slivers
      PE  : s += qj^T @ rinv -- K=128 M=1 reduce matmuls, 4-way COL-TILED
            (tile_position=(0,32c)) so they run concurrently; the four
            accumulator rows live at partitions 0/32/64/96 of one PSUM
            bank and the host sums them.
    s is copied to SBUF and DMA'd out on four queues in parallel.
  * HOST does everything sparse/small in float64: the close-pair (d<3)
    erf correction, the half-window (delta=N/2) pairs, the DIAG_EPS
    mirror, the final q_i . s dot and the 8-core sum.
  * A dummy ACTIVATE is the first Scalar-queue instruction so the
    ACT_TABLE_LOADs run during the input DMAs; Scalar issues no DMAs.
"""

import numpy as np

P = 128
N = 4096
NCORES = 8
COLS = N // NCORES          # 512 i-columns per core
HALF = N // 2               # cyclic half window (2048)
WIN = HALF + COLS           # 2560 j-rows per core
NB = WIN // P               # 20 j-blocks per core
K = 16                      # split-matmul contraction rows
NORM_CONST = 90.0474 / (2.0 * np.pi)
DIAG_EPS = 0.01             # d2 bias on same-window (u<512) j-rows
RCUT2 = 9.0                 # close-pair cutoff d^2 (d < 3)

# 5 super-tiles (blocks sharing one PSUM tile / one ACTIVATE), sizes
# alternating 1536 (3 banks, pool A) / 2048 (4 banks, pool B) so adjacent
# supers fit in 7 banks and the s accumulator keeps the 8th.
# Masked edge blocks all land in the first two supers -> tail is clean.
SUPERS = [[0, 1, 2, 3, 18], [16, 17, 19, 4, 5], [6, 7, 8],
          [9, 10, 11, 12], [13, 14, 15]]
_BLOCK_ORDER = [k for blocks in SUPERS for k in blocks]
_BLOCK_POS = {k: pos for pos, k in enumerate(_BLOCK_ORDER)}

_CACHE = {}


def _block_geom(k):
    """Active i-col range [vlo, vlo+w) of j-block k in the 512-wide window."""
    if k < 4:                       # leading: v in [0, 128(k+1))
        return 0, P * (k + 1)
    if k < 16:                      # full
        return 0, COLS
    kk = k - 16                     # trailing: v in [128kk, 512)
    return P * kk, COLS - P * kk


def _super_layout(blocks):
    offs, off = [], 0
    for k in blocks:
        _, w = _block_geom(k)
        offs.append(off)
        off += w
    return offs, off


def _split2(v32):
    h = v32.astype(np.float16)
    l = (v32 - h.astype(np.float32)).astype(np.float16)
    return h, l


def _build_core_inputs(q, r):
    q = q.astype(np.float32)
    r = r.astype(np.float32)
    r2_64 = (r.astype(np.float64) ** 2).sum(1)

    iota = np.broadcast_to(np.arange(P, dtype=np.float16), (P, P))
    thr = np.arange(P, dtype=np.float16)[:, None]

    in_maps = []
    for c in range(NCORES):
        perm = (COLS * c + np.arange(WIN)) % N      # j-slot u -> atom index
        win = slice(COLS * c, COLS * (c + 1))       # this core's i-window

        rows_j, rows_i = [], []
        for d in range(3):
            jh, jl = _split2(r[perm, d])
            ih, il = _split2((-2.0 * r[win, d]).astype(np.float32))
            rows_j += [jh, jh, jl, jl]
            rows_i += [ih, il, ih, il]
        r2j = r2_64[perm].copy()
        r2j[:COLS] += DIAG_EPS                      # same-window bias
        jh, jl = _split2(r2j.astype(np.float32))
        ih, il = _split2(r2_64[win].astype(np.float32))
        ones_j = np.ones(WIN, np.float16)
        ones_i = np.ones(COLS, np.float16)
        rows_j += [jh, jl, ones_j, ones_j]
        rows_i += [ones_i, ones_i, ih, il]
        aj_nat = np.stack(rows_j).astype(np.float16)    # [16, WIN] natural order
        # reorder j-blocks into SUPER order so the first DMA chunk covers S0
        in_aj = np.empty_like(aj_nat)
        for pos, k in enumerate(_BLOCK_ORDER):
            in_aj[:, P * pos:P * (pos + 1)] = aj_nat[:, P * k:P * (k + 1)]
        in_bi = np.stack(rows_i).astype(np.float16)     # [16, COLS]

        qjw = np.zeros((P, NB), np.float16)
        for k in range(NB):
            qjw[:, k] = q[perm[P * k:P * (k + 1)]].astype(np.float16)
        in2 = np.concatenate(
            [qjw, iota, thr, np.zeros((P, 11), np.float16)], axis=1)  # [128,160]
        in_maps.append({"in_aj": in_aj, "in_bi": in_bi, "in2": in2})
    return in_maps


def _host_correction(q, r):
    """Sparse correction in float64: for every pair the main term got wrong
    (close pairs d<3 and the delta=N/2 pairs the strict masks exclude),
    add  C*qq*( erf(d/sqrt2)/d  -  main_term_contribution )."""
    from scipy.spatial import cKDTree
    from scipy.special import erf

    r = np.asarray(r, dtype=np.float64)
    q = np.asarray(q, dtype=np.float64)

    pairs = cKDTree(r).query_pairs(np.sqrt(RCUT2), output_type='ndarray')
    pset = set()
    if len(pairs):
        a, b = pairs[:, 0].astype(np.int64), pairs[:, 1].astype(np.int64)
        delta = (b - a) % N
        flip = delta > HALF
        aa = np.where(flip, b, a)
        bb = np.where(flip, a, b)
        pset.update(zip(aa.tolist(), bb.tolist()))
    # delta = N/2 pairs are excluded from the main window by the strict masks
    for i in range(HALF):
        pset.add((i, i + HALF))
    ia = np.array([p[0] for p in pset], dtype=np.int64)
    ib = np.array([p[1] for p in pset], dtype=np.int64)

    d2 = ((r[ia] - r[ib]) ** 2).sum(1)
    dist = np.sqrt(d2)
    g = np.where(d2 > 0, erf(dist / np.sqrt(2.0)) / np.maximum(dist, 1e-300),
                 np.sqrt(2.0 / np.pi))
    delta = (ib - ia) % N
    v = ia % COLS
    in_main = delta < HALF                       # delta==HALF excluded by masks
    sgE = ((v + delta) < COLS) & in_main         # DIAG_EPS mirror
    main_got = np.where(in_main, 1.0 / np.sqrt(d2 + np.where(sgE, DIAG_EPS, 0.0)),
                        0.0)
    qq = q[ia] * q[ib]
    return NORM_CONST * np.sum(qq * (g - main_got))


def _build_program():
    import concourse.mybir as mybir
    import concourse.tile as tile
    from concourse import bacc

    dt = mybir.dt
    alu = mybir.AluOpType
    rsq_fn = mybir.ActivationFunctionType.Abs_reciprocal_sqrt
    nc = bacc.Bacc("TRN2", target_bir_lowering=False, debug=False,
                   num_devices=NCORES)

    in_aj = nc.dram_tensor("in_aj", [K, WIN], dt.float16, kind="ExternalInput")
    in_bi = nc.dram_tensor("in_bi", [K, COLS], dt.float16, kind="ExternalInput")
    in2 = nc.dram_tensor("in2", [P, 160], dt.float16, kind="ExternalInput")
    s_out = nc.dram_tensor("s", [4, COLS], dt.float32, kind="ExternalOutput")

    with tile.TileContext(nc) as tc:
        with (
            tc.tile_pool(name="const", bufs=1) as cpool,
            tc.tile_pool(name="rinv", bufs=2) as rpool,
            tc.tile_pool(name="d2a", bufs=1, space="PSUM") as ppoolA,
            tc.tile_pool(name="d2b", bufs=1, space="PSUM") as ppoolB,
            tc.tile_pool(name="sacc", bufs=1, space="PSUM") as spool,
        ):
            # ---- dummy ACTIVATE first: pulls the ACT_TABLE_LOADs to t=0
            scr = cpool.tile([1, 16], dt.float32)
            nc.vector.memset(scr[:], 1.0)
            dum = cpool.tile([1, 16], dt.float32)
            nc.scalar.activation(dum[:], scr[:], rsq_fn)

            # ---- inputs (never on the Scalar queue)
            AJF = cpool.tile([K, WIN], dt.float16)
            BIF = cpool.tile([K, COLS], dt.float16)
            IN2 = cpool.tile([P, 160], dt.float16)
            CUT = P * len(SUPERS[0])         # first super's stationaries
            nc.sync.dma_start(AJF[:, :CUT], in_aj[:, :CUT])
            nc.gpsimd.dma_start(BIF[:], in_bi[:])
            nc.sync.dma_start(AJF[:, CUT:], in_aj[:, CUT:])
            nc.gpsimd.dma_start(IN2[:], in2[:])
            QJW = IN2[:, :NB]
            IOTA = IN2[:, NB:NB + P]
            THR = IN2[:, NB + P:NB + P + 1]

            # ---- warm matmuls + zero seed of the s accumulator
            wl = cpool.tile([1, 1], dt.float16)
            wsrc = cpool.tile([1, COLS], dt.float16)
            zrow = cpool.tile([1, P], dt.float16)
            nc.vector.memset(wl[:], 0.0)
            nc.vector.memset(wsrc[:], 0.0)
            nc.vector.memset(zrow[:], 0.0)
            S = spool.tile([P, COLS], dt.float32)
            for _ in range(3):
                nc.tensor.matmul(S[0:1, :], wl[:, :], wsrc[:, :],
                                 start=True, stop=True, skip_group_check=True)
            # K=1, M=128 zero matmul seeds all four accumulator rows
            nc.tensor.matmul(S[:, :], zrow[:, :], wsrc[:, :],
                             start=True, stop=False, skip_group_check=True)

            d2_tiles = [None] * len(SUPERS)
            rinv_tiles = [None] * len(SUPERS)

            def emit_d2(si):
                blocks = SUPERS[si]
                offs, W = _super_layout(blocks)
                pool = ppoolA if si % 2 == 0 else ppoolB
                d2 = pool.tile([P, W], dt.float32, tag=f"d2{si % 2}")
                for k, off in zip(blocks, offs):
                    vlo, w = _block_geom(k)
                    x = 0
                    while x < w:   # split at PSUM bank boundaries
                        hi = min(w, ((off + x) // COLS + 1) * COLS - off)
                        pk = _BLOCK_POS[k]
                        nc.tensor.matmul(
                            d2[:, off + x:off + hi],
                            AJF[:, P * pk:P * (pk + 1)],
                            BIF[:, vlo + x:vlo + hi],
                            start=True, stop=True, skip_group_check=True,
                        )
                        x = hi
                d2_tiles[si] = d2

            def emit_act(si):
                _, W = _super_layout(SUPERS[si])
                rinv = rpool.tile([P, 2048], dt.float16, tag="rinv")
                nc.scalar.activation(rinv[:, :W], d2_tiles[si][:, :W], rsq_fn)
                rinv_tiles[si] = rinv

            def emit_masks(si):
                blocks = SUPERS[si]
                offs, _ = _super_layout(blocks)
                rinv = rinv_tiles[si]
                for k, off in zip(blocks, offs):
                    _, w = _block_geom(k)
                    if k < 4:        # leading: mask last 128 cols, keep v' < p
                        sl = slice(off + w - P, off + w)
                        op = alu.is_lt
                    elif k >= 16:    # trailing: mask first 128 cols, keep v' > p
                        sl = slice(off, off + P)
                        op = alu.is_gt
                    else:
                        continue
                    nc.vector.scalar_tensor_tensor(
                        out=rinv[:, sl], in0=IOTA[:, :], scalar=THR[:, :],
                        in1=rinv[:, sl], op0=op, op1=alu.mult)

            def emit_reduce(si, last=False):
                blocks = SUPERS[si]
                offs, _ = _super_layout(blocks)
                rinv = rinv_tiles[si]
                for j, (k, off) in enumerate(zip(blocks, offs)):
                    vlo, w = _block_geom(k)
                    cg = 32 * (j % 4)      # col-tiled: concurrent reduce MMs
                    nc.tensor.matmul(
                        S[cg:cg + 1, vlo:vlo + w],
                        QJW[:, k:k + 1],
                        rinv[:, off:off + w],
                        start=False, stop=last and j == len(blocks) - 1,
                        skip_group_check=True,
                        tile_position=(0, cg),
                    )

            # ---- pipeline (d2 runs two supers ahead of its reduce) ----
            emit_d2(0); emit_d2(1)
            emit_act(0); emit_masks(0); emit_d2(2)
            emit_act(1); emit_masks(1); emit_d2(3); emit_reduce(0)
            emit_act(2); emit_d2(4); emit_reduce(1)
            emit_act(3); emit_reduce(2)
            emit_act(4); emit_reduce(3)
            emit_reduce(4, last=True)

            s_sb = cpool.tile([P, COLS], dt.float32)
            nc.vector.tensor_copy(s_sb[:], S[:])
            nc.sync.dma_start(s_out[:, :], s_sb[0:97:32, :])

    nc.compile()
    return nc


def _get_program():
    if "nc" not in _CACHE:
        _CACHE["nc"] = _build_program()
    return _CACHE["nc"]


def _run(q, r, trace=False, **trace_kwargs):
    from concourse.bass_utils import run_bass_kernel_spmd

    q = np.asarray(q)
    r = np.asarray(r)
    nc = _get_program()
    in_maps = _build_core_inputs(q, r)
    res = run_bass_kernel_spmd(nc, in_maps, core_ids=list(range(NCORES)),
                               trace=trace, **trace_kwargs)
    q64 = np.asarray(q, dtype=np.float64)
    total = np.float64(0.0)
    for c, m in enumerate(res.results):
        s = m["s"].astype(np.float64).sum(0)        # sum the 4 col-group rows
        total += NORM_CONST * np.dot(q64[COLS * c:COLS * (c + 1)], s)
    total += _host_correction(q, r)
    return np.array([total], dtype=np.float32), res


def kernel(q, r, cell=None, batch=None):
    out, _ = _run(q, r, trace=False)
    return out
